# revision 34
# baseline (speedup 1.0000x reference)
"""Balanced-softmax loss (BSLClassifier) on 8 Trainium2 NeuronCores.

loss = -(1/B) * sum_b [ pred[b,t_b] + log(freq[t_b]) - log(sum_c exp(pred[b,c])*freq[c]) ]

Strategy: data-parallel over batch B. Per core the shard is laid out
class-major ([C=1000, Bc=4096]) and the batch columns are SORTED by
target class (host-side layout; the loss is permutation-invariant):
  - ACT : exp(pred_T + logfreq) in one op per 128-class chunk --
          logfreq[c] is constant per partition, so it rides the
          activation's per-partition bias. Output expT bf16.
  - PE  : rsum[b] = sum_c exp(...) via ones-vector matvecs in bf16,
          accumulating the 8 class chunks in PSUM (fp32).
  - DVE : picked = sum_b pred_T[t_b, b] via one fused
          scalar_tensor_tensor per chunk, restricted to the (sorted,
          contiguous, host-known) column range whose targets fall in
          that chunk -- ~1/8 of the columns each, so the gather is
          nearly free. Also copies PSUM->SBUF for the output.
  - host: histogram, sort, tiny log/sum finalization in f64.

pred is read exactly once from HBM; ACT (exp) and DMA set the roofline.
The program is rebuilt if the targets change (chunk column ranges are
compile-time constants).
"""

import hashlib

import numpy as np
import ml_dtypes

B, C = 32768, 1000
NCORES = 8
BC = B // NCORES    # 4096 batch columns per core
P = 128             # partitions
NK = (C + P - 1) // P  # 8 class chunks (last one 104 rows)
NJ = BC // 512      # 8 psum column blocks per core

_CACHE = {}


def _split_multi_waits(nc, max_waits=1):
    """This container's walrus build accepts at most one sync-wait per
    instruction; Tile emits several. Split extras into standalone
    EventSemaphore instructions on the same engine, immediately before."""
    from concourse import mybir

    n_new = 0
    for func in nc.m.functions:
        for bb in func.blocks:
            out = []
            changed = False
            for ins in bb.instructions:
                si = ins.sync_info
                if si is not None and len(si.on_wait) > max_waits:
                    waits = list(si.on_wait)
                    extra, keep = waits[:-max_waits], waits[-max_waits:]
                    for w in extra:
                        n_new += 1
                        ev = mybir.InstEventSemaphore(
                            name=f"wsplit_{n_new}", ins=[], outs=[]
                        )
                        ev.engine = ins.engine
                        ev.sync_info = mybir.SyncInfo(on_update=[], on_wait=[w])
                        out.append(ev)
                    ins.sync_info = mybir.SyncInfo(
                        on_update=list(si.on_update), on_wait=keep
                    )
                    changed = True
                out.append(ins)
            if changed:
                bb.instructions = out
    return n_new


def _build_bass(ranges):
    """ranges[k] = (off, n): column range per class chunk, identical
    layout on every core (host pads/aligns them)."""
    import concourse.bass as bass
    import concourse.tile as tile
    from concourse import mybir

    f32 = mybir.dt.float32
    bf16 = mybir.dt.bfloat16
    i16 = mybir.dt.int16
    Alu = mybir.AluOpType
    Act = mybir.ActivationFunctionType

    nc = bass.Bass()
    predt = nc.dram_tensor("predt", [C, BC], bf16, kind="ExternalInput")
    lfcol = nc.dram_tensor("lfcol", [P, NK], f32, kind="ExternalInput")
    tbc = nc.dram_tensor("tbc", [1, BC], i16, kind="ExternalInput")
    iotac = nc.dram_tensor("iotac", [P, NK], i16, kind="ExternalInput")
    onesb = nc.dram_tensor("onesb", [P, 1], bf16, kind="ExternalInput")
    rsum = nc.dram_tensor("rsum", [1, BC], f32, kind="ExternalOutput")
    picked = nc.dram_tensor("picked", [P, NK], f32, kind="ExternalOutput")

    with tile.TileContext(nc) as tc:
        with (
            tc.tile_pool(name="const", bufs=1) as const_pool,
            tc.tile_pool(name="io", bufs=5) as io_pool,
            tc.tile_pool(name="work", bufs=3) as work_pool,
            tc.tile_pool(name="ps", bufs=1, space="PSUM") as psum_pool,
            tc.tile_pool(name="acc", bufs=1) as acc_pool,
        ):
            picked_acc = acc_pool.tile([P, NK], f32)
            nc.vector.memset(picked_acc, 0.0)
            # one bank per 512-column block, all on partition 0
            rsum_ps = psum_pool.tile([1, NJ, 512], f32)

            # tiny constants first (exp0 needs lf; don't let it queue
            # behind megabyte chunk transfers), then chunk prefetches;
            # chunk 0 in column halves across both HWDGE rings
            lf_t = const_pool.tile([P, NK], f32)
            nc.sync.dma_start(out=lf_t, in_=lfcol[:])
            ones_t = const_pool.tile([P, 1], bf16)
            nc.sync.dma_start(out=ones_t, in_=onesb[:])
            iota_t = const_pool.tile([P, NK], i16)
            nc.scalar.dma_start(out=iota_t, in_=iotac[:])

            H = BC // 2

            def load_chunk(k, pk, split):
                pt = io_pool.tile([P, BC], bf16, tag="ptile")
                if split:
                    nc.sync.dma_start(
                        out=pt[:pk, 0:H], in_=predt[k * P : k * P + pk, 0:H]
                    )
                    nc.scalar.dma_start(
                        out=pt[:pk, H:BC], in_=predt[k * P : k * P + pk, H:BC]
                    )
                else:
                    eng = nc.sync if k % 2 == 0 else nc.scalar
                    eng.dma_start(
                        out=pt[:pk], in_=predt[k * P : k * P + pk, :]
                    )
                return pt

            ptiles = {}
            for k in range(4):
                ptiles[k] = load_chunk(k, min(P, C - k * P), split=k < 2)
            tbc_t = const_pool.tile([P, BC], i16)
            tbc_row = tbc[0, :]
            tbc_bcast = bass.AP(
                tensor=tbc_row.tensor,
                offset=tbc_row.offset,
                ap=[[0, P], [1, BC]],
            )
            nc.scalar.dma_start(out=tbc_t, in_=tbc_bcast)

            rsum_sb = acc_pool.tile([1, BC], f32)

            for k in range(NK):
                pk = min(P, C - k * P)  # 104 on the last chunk
                if k in ptiles:
                    ptile = ptiles[k]
                else:
                    ptile = load_chunk(k, pk, split=False)

                expt = work_pool.tile([P, BC], bf16, tag="expt")
                if k < 2 or k == NK - 1:
                    nc.scalar.activation(
                        expt[:pk, 0:H], ptile[:pk, 0:H], Act.Exp,
                        bias=lf_t[:pk, k : k + 1],
                    )
                    nc.scalar.activation(
                        expt[:pk, H:BC], ptile[:pk, H:BC], Act.Exp,
                        bias=lf_t[:pk, k : k + 1],
                    )
                else:
                    nc.scalar.activation(
                        expt[:pk], ptile[:pk], Act.Exp, bias=lf_t[:pk, k : k + 1]
                    )

                for j in range(NJ):
                    nc.tensor.matmul(
                        rsum_ps[0:1, j, :],
                        ones_t[:pk],
                        expt[:pk, j * 512 : (j + 1) * 512],
                        start=(k == 0),
                        stop=(k == NK - 1),
                    )

                off, n = ranges[k]
                if n > 0:
                    scr = work_pool.tile([P, BC], bf16, tag="scr")
                    nc.vector.scalar_tensor_tensor(
                        out=scr[:pk, 0:n],
                        in0=tbc_t[:pk, off : off + n],
                        scalar=iota_t[:pk, k : k + 1],
                        in1=ptile[:pk, off : off + n],
                        op0=Alu.is_equal,
                        op1=Alu.mult,
                        accum_out=picked_acc[:pk, k : k + 1],
                    )

            for j in range(NJ):
                if j % 2 == 0:
                    nc.vector.tensor_copy(
                        rsum_sb[0:1, j * 512 : (j + 1) * 512], rsum_ps[0:1, j, :]
                    )
                else:
                    nc.scalar.copy(
                        rsum_sb[0:1, j * 512 : (j + 1) * 512], rsum_ps[0:1, j, :]
                    )
            nc.sync.dma_start(out=rsum[:], in_=rsum_sb)
            nc.sync.dma_start(out=picked[:], in_=picked_acc)

    _split_multi_waits(nc)
    return nc


def kernel(pred, target):
    from concourse.bass_utils import run_bass_kernel_spmd

    pred = np.asarray(pred)
    target = np.asarray(target)
    tgt64 = target.astype(np.int64)
    assert pred.shape == (B, C) and tgt64.shape == (B,)

    # host-side tiny index math
    freq = np.bincount(tgt64, minlength=C).astype(np.float64)
    logfreq = np.where(freq > 0, np.log(np.maximum(freq, 1.0)), -30000.0)
    lf32 = logfreq.astype(np.float32)
    lfcol = np.zeros((P, NK), dtype=np.float32)
    iotac = np.zeros((P, NK), dtype=np.int16)
    for k in range(NK):
        pk = min(P, C - k * P)
        lfcol[:pk, k] = lf32[k * P : k * P + pk]
        iotac[:pk, k] = np.arange(k * P, k * P + pk, dtype=np.int16)
    onesb = np.ones((P, 1), dtype=ml_dtypes.bfloat16)

    # per-core batch sort by target class; shared padded chunk ranges
    orders = []
    counts = np.zeros((NCORES, NK), dtype=np.int64)
    for c in range(NCORES):
        tc_ = tgt64[c * BC : (c + 1) * BC]
        order = np.argsort(tc_, kind="stable")
        orders.append(order)
        counts[c] = np.bincount(tc_ // P, minlength=NK)
    # one shared range table (compile-time): pad each chunk's width to the
    # max across cores; offsets by cumulative max widths (fits: sum of
    # maxima <= BC + slack is not guaranteed, so clamp via per-core offsets
    # baked per chunk -- instead use per-chunk max width and overlapping is
    # fine because we place each core's chunk block at its own offset and
    # scan [min_off, max_end). Simpler: scan range = [min_off, max_end).
    offs = np.zeros((NCORES, NK + 1), dtype=np.int64)
    for c in range(NCORES):
        offs[c, 1:] = np.cumsum(counts[c])
    ranges = []
    for k in range(NK):
        lo = int(offs[:, k].min())
        hi = int(offs[:, k + 1].max())
        ranges.append((lo, hi - lo))

    key = ("nc", hashlib.sha1(tgt64.tobytes()).hexdigest())
    if _CACHE.get("key") != key:
        _CACHE["nc"] = _build_bass(ranges)
        _CACHE["key"] = key
    nc = _CACHE["nc"]

    in_maps = []
    for c in range(NCORES):
        sl = slice(c * BC, (c + 1) * BC)
        order = orders[c]
        predt_c = np.ascontiguousarray(
            pred[sl][order].astype(ml_dtypes.bfloat16).T
        )
        tbc_c = np.ascontiguousarray(
            tgt64[sl][order].astype(np.int16).reshape(1, BC)
        )
        in_maps.append(
            {
                "predt": predt_c,
                "lfcol": lfcol,
                "tbc": tbc_c,
                "iotac": iotac,
                "onesb": onesb,
            }
        )

    res = run_bass_kernel_spmd(nc, in_maps, core_ids=list(range(NCORES)))
    _CACHE["last_results"] = res

    # host-side final reduction in f64 (tiny)
    # picked sums pred[b, t_b] (fp32 accumulate of bf16 pred values);
    # rsum[b] = sum_c exp(pred + logfreq)
    s = 0.0
    s += logfreq[tgt64].sum()  # sum_b log(freq[t_b])
    lastpk = C - (NK - 1) * P
    for c in range(NCORES):
        out = res.results[c]
        pk_arr = out["picked"].astype(np.float64)
        s += pk_arr[:, : NK - 1].sum() + pk_arr[:lastpk, NK - 1].sum()
        s -= np.log(out["rsum"].astype(np.float64)).sum()
    return np.asarray(-s / B, dtype=np.float32)
